# revision 96
# baseline (speedup 1.0000x reference)
"""ConvSquare Trainium2 kernel.

Math: out = conv2d_3x3(x * poly(alpha), weight) + bias, stride 1, pad 1,
where poly(t) = (a*t + b)*t + c applied to the zero-padded alpha field.
(The reference's unfold/einsum collapses to this because x is zero-padded:
border window positions contribute x=0 regardless of the kernel value.)

Sharding: 8 cores = batch(4) x row-half(2). Each core computes a
[O=64, 64, 128] output slab from a zero-padded [C=64, 66, 130] input slab.

Design (final, ~30.3us modeled vs 51.5us baseline):
- bf16 x/y/w/out, fp16 alpha/z; host converts the output back to f32.
- one DMA per chunk loads [x_j | alpha_j] (bf16 container; alpha is fp16
  bits) from a host-packed image. The first DL chunks are double-loaded:
  a 3-dim AP (outer dim stride WP over the DRAM image) fills partitions
  64-127 with the +1-row-shifted window, so the elementwise pass below
  produces y AND its row-shifted replica at once (engine cost depends
  only on free size). Later chunks get the replica via an SBUF shift copy.
- poly via completed square: kal = sgn*(s*al + t)^2 + d -> one ACT Square
  (z, fp16) + one DVE scalar_tensor_tensor (y = (z -/+ d)*x, bf16); the
  sign is folded into host-negated weights. Two-op feed chain per chunk.
- 5 matmuls per 512-col output chunk instead of 9 naive: 1 single (2,2)
  + 3 row pairs ((0,l)+(1,l) via y1's shifted upper half) + 1 col pair
  ((2,0)+(2,1) via y2 = [y ; y shifted one column]).
- queue balance: inputs/outputs/shift/y2-lower on SP's HWDGE queue, ycol
  and the last output group on the Pool SWDGE queue (HWDGE gen ~0.63us
  and Pool gen ~1.0us serialize per queue; balance beats either alone).
- a p-state bridge matmul on a const tile at t~0.8us starts the PE ramp
  clock early and const-based warmups keep PE busy through the prologue,
  so all real matmuls are costed at full clock (idle-start would halve
  the clock for the first ~3us of visits).
- a dummy activation at t=0 pins LoadActFuncSet off the critical path.
- bias-add + PSUM->SBUF downcast on ACT (Identity + bias AP); the last
  chunk's bias runs on DVE (tracks the stop-matmul within ~100ns).
- output DMA groups of [6,6,2,1,1] chunks so the final transfer after
  the last matmul is minimal.
"""

import sys

import ml_dtypes
import numpy as np

sys.path.insert(0, "/opt/trn_rl_repo")

import concourse.bass as bass
import concourse.mybir as mybir
from concourse.bass_utils import run_bass_kernel_spmd
from concourse.tile import TileContext

F32 = mybir.dt.float32
BF16 = mybir.dt.bfloat16
FP16 = mybir.dt.float16
NPBF16 = ml_dtypes.bfloat16

B, C, O, H, W = 4, 64, 64, 128, 128
HS = 64  # output rows per core
RP = HS + 2  # padded input rows (66)
WP = W + 2  # padded cols (130)
FREE = RP * WP  # 8580
EW_ROWS = [(0, 6), (6, 11), (11, 16), (16, 22), (22, 30), (30, 38), (38, 46),
           (46, 56), (56, 66)]
EW_N = [(r1 - r0) * WP for r0, r1 in EW_ROWS]
DL = 4  # first DL chunks are double-loaded (128 partitions, built-in shift)
# sbuf region len per chunk: double-loaded = [x|x-extra-row|al], else [x|al]
EW_L = [2 * n + WP if j < DL else 2 * n for j, n in enumerate(EW_N)]
EW_HOST = [2 * (n + WP) if j < DL else 2 * n for j, n in enumerate(EW_N)]
XL = sum(EW_L)
XL_HOST = sum(EW_HOST)
NCHUNK = 16  # matmul blocks (4 out rows each)
MM_N = 4 * W  # 512
OGROUPS = [6, 6, 2, 1, 1]  # output DMA groups (psum chunks each)
# col-pair of block i is emitted in block CP_AT[i]: deferred two blocks
# early on (its y2 copies land late), in-block once the copies lead the PE
CP_AT = {i: (i + 2 if i < 6 else (i + 1 if i < 10 else i))
         for i in range(NCHUNK)}
WARM0 = 2  # warmup matmuls (the scheduler hoists them to the prologue)
WARM_IN = {}

_cache: dict = {}


def _program(av: float, bv: float, cv: float) -> bass.Bass:
    from concourse.bacc import Bacc

    nc = Bacc()
    xa_h = nc.dram_tensor("xa", [64, XL_HOST], BF16, kind="ExternalInput")
    w_h = nc.dram_tensor("w", [128, 320], BF16, kind="ExternalInput")
    bias_h = nc.dram_tensor("bias", [O, 1], F32, kind="ExternalInput")
    out_h = nc.dram_tensor("out", [O, HS * W], BF16, kind="ExternalOutput")

    # poly(alpha) = a*al^2 + b*al + c as sgn*(s*al + t)^2 + d
    if av == 0.0:
        # linear fallback: z = b*al + c (ACT Copy), y = (z + 0) * x
        s, t, d = bv, cv, 0.0
        op0 = mybir.AluOpType.add
    elif av < 0:
        s = (-av) ** 0.5
        t = -bv / (2 * s)
        d = cv - bv * bv / (4 * av)
        op0 = mybir.AluOpType.subtract  # y = (z - d)*x = -(kal)*x, w negated
    else:
        s = av ** 0.5
        t = bv / (2 * s)
        d = cv - bv * bv / (4 * av)
        op0 = mybir.AluOpType.add  # y = (z + d)*x

    def mk_ap(base, offset, dims):
        return bass.AP(tensor=base.tensor, offset=offset, ap=dims)

    with TileContext(nc) as tc:
        with (
            tc.tile_pool(name="const", bufs=1) as cpool,
            tc.tile_pool(name="work", bufs=1) as wpool,
            tc.tile_pool(name="outs", bufs=5) as opool,
            tc.tile_pool(name="psum", bufs=6, space="PSUM") as ppool,
            tc.tile_pool(name="wpsum", bufs=1, space="PSUM") as wppool,
        ):
            wt = cpool.tile([128, 320], BF16)
            bt = cpool.tile([O, 1], F32)
            tc_b = cpool.tile([128, 1], F32)
            nc.gpsimd.memset(tc_b[:, :], t)
            cb2 = cpool.tile([128, 1], BF16)
            nc.gpsimd.memset(cb2[:, :], 0.5)
            zdum = cpool.tile([128, 1], F32)
            # dummy op: pins LoadActFuncSet at t~0, off the critical path
            nc.scalar.activation(
                zdum[:, :], tc_b[:, :],
                mybir.ActivationFunctionType.Square, bias=tc_b[:, 0:1],
                scale=1.0,
            )

            xa = wpool.tile([128, XL], BF16)
            zt = wpool.tile([128, FREE], FP16)
            y1 = wpool.tile([128, FREE], BF16)
            y2 = wpool.tile([128, FREE], BF16)

            # SP/HWDGE queue: all loads up-front (no waits -> no blocking)
            so_j = [0]
            for L in EW_L:
                so_j.append(so_j[-1] + L)
            ho_j = [0]
            for L in EW_HOST:
                ho_j.append(ho_j[-1] + L)
            for j in range(len(EW_ROWS)):
                if j < DL:
                    # 3-dim AP: outer dim of stride WP loads partitions
                    # 64-127 with the +1-row shifted window
                    nc.sync.dma_start(
                        out=xa[:, so_j[j]:so_j[j + 1]],
                        in_=mk_ap(xa_h[:, :], ho_j[j],
                                  [[WP, 2], [XL_HOST, 64], [1, EW_L[j]]]),
                    )
                else:
                    nc.sync.dma_start(
                        out=xa[0:64, so_j[j]:so_j[j + 1]],
                        in_=xa_h[:, ho_j[j]:ho_j[j + 1]],
                    )
                if j == 1:
                    nc.sync.dma_start(out=wt[:, :], in_=w_h[:, :])
                    nc.sync.dma_start(out=bt[:, :], in_=bias_h[:, :])

            wps = wppool.tile([128, 512], F32)

            # p-state bridge: one long f32 matmul on the const tile (ready at
            # ~0.5us) marks the PE busy-period start early, so the real
            # matmuls are costed at full clock. f32 runs 4 cycles/row; the
            # stride-0 rhs makes it 512 rows from a [128,1] tile.
            nc.tensor.matmul(
                wps[0:1, :], tc_b[:, 0:1], tc_b[:, 0:1].to_broadcast((128, 512)),
                start=True, stop=True, skip_group_check=True,
            )
            nc.tensor.matmul(
                wps[0:1, 0:256], tc_b[:, 0:1],
                tc_b[:, 0:1].to_broadcast((128, 256)),
                start=True, stop=True, skip_group_check=True,
            )

            def warmup(k):
                # const-tile operands: no input dependency, can run at t~1us
                for _ in range(k):
                    nc.tensor.matmul(
                        wps[0:1, 0:320], cb2[:, 0:1],
                        cb2[:, 0:1].to_broadcast((128, 320)),
                        start=True, stop=True, skip_group_check=True,
                    )

            def emit_poly(j, r0, r1):
                R0 = EW_ROWS[j][0]
                n = EW_N[j]
                off = (r0 - R0) * WP
                m = (r1 - r0) * WP
                sl = slice(r0 * WP, r1 * WP)
                if j < DL:
                    np_, al0 = 128, so_j[j] + n + WP
                else:
                    np_, al0 = 64, so_j[j] + n
                x_ap = xa[0:np_, so_j[j] + off:so_j[j] + off + m]
                al_ap = xa[0:np_, al0 + off:al0 + off + m].bitcast(FP16)
                # z = (s*al + t)^2   (ACT Square, fp16); linear for a=0
                if av == 0.0:
                    nc.scalar.activation(
                        zt[0:np_, sl], al_ap,
                        mybir.ActivationFunctionType.Copy, bias=t, scale=s,
                    )
                else:
                    nc.scalar.activation(
                        zt[0:np_, sl], al_ap,
                        mybir.ActivationFunctionType.Square,
                        bias=tc_b[0:np_, 0:1], scale=s,
                    )
                # y = (z -/+ d) * x  (DVE scalar_tensor_tensor, bf16)
                # double-loaded chunks produce y and its row-shifted replica
                # in the same pass (partitions 64-127)
                nc.vector.scalar_tensor_tensor(
                    out=y1[0:np_, sl], in0=zt[0:np_, sl], in1=x_ap, scalar=d,
                    op0=op0, op1=mybir.AluOpType.mult,
                )

            def emit_shift(j):
                r0, r1 = EW_ROWS[j]
                a0, b0 = r0 * WP, r1 * WP
                if j >= DL:
                    # +1-row shift onto y1 upper (SP queue)
                    c0, c1 = max(0, a0 - WP), b0 - WP
                    nc.sync.dma_start(
                        out=y1[64:128, c0:c1], in_=y1[0:64, c0 + WP:c1 + WP]
                    )

            def emit_y2(r0, r1):
                # SP queue: y2 = [y ; y shifted one column]
                a0, b0 = r0 * WP, r1 * WP
                nc.sync.dma_start(out=y2[0:64, a0:b0], in_=y1[0:64, a0:b0])
                d0, d1 = max(0, a0 - 1), b0 - 1
                nc.gpsimd.dma_start(
                    out=y2[64:128, d0:d1], in_=y1[0:64, d0 + 1:d1 + 1]
                )

            def emit_chunk(j):
                emit_poly(j, *EW_ROWS[j])
                emit_shift(j)
                emit_y2(*EW_ROWS[j])

            y1_3 = y1[:].rearrange("p (r c) -> p r c", r=RP)
            y2_3 = y2[:].rearrange("p (r c) -> p r c", r=RP)

            # chunk 0 split: rows 2-6 first (block 0's single reads rows 2-5)
            emit_poly(0, 2, 6)
            emit_poly(0, 0, 2)
            emit_y2(*EW_ROWS[0])
            warmup(WARM0)
            for k in range(1, len(EW_ROWS)):
                emit_chunk(k)

            psums = {}
            gi0 = 0
            g = 0
            ot = None

            def colpair_and_bias(i2):
                nonlocal gi0, g, ot
                ps2 = psums.pop(i2)
                nc.tensor.matmul(
                    ps2[:].rearrange("p (r c) -> p r c", r=4),
                    wt[0:128, 256:320],
                    y2_3[0:128, 4 * i2 + 2:4 * i2 + 6, 0:W],
                    start=False, stop=True, skip_group_check=True,
                )
                if i2 == gi0:
                    ot = opool.tile([O, OGROUPS[g] * MM_N], BF16)
                oc = (i2 - gi0) * MM_N
                if i2 >= NCHUNK - 1:
                    # tail-critical biases on DVE: idle there, and it starts
                    # within ~100ns of the stop-matmul firing
                    nc.vector.tensor_scalar(
                        out=ot[:, oc:oc + MM_N], in0=ps2[:, :],
                        scalar1=bt[:, 0:1], scalar2=None,
                        op0=mybir.AluOpType.add,
                    )
                else:
                    nc.scalar.activation(
                        ot[:, oc:oc + MM_N], ps2[:, :],
                        mybir.ActivationFunctionType.Identity,
                        bias=bt[:, 0:1], scale=1.0,
                    )
                if i2 - gi0 == OGROUPS[g] - 1:
                    # last two groups issue from the (idle) ACT/DVE queues so
                    # the earlier groups' DMAs can't head-of-line block them
                    eng = nc.sync
                    eng.dma_start(
                        out=out_h[:, gi0 * MM_N:(gi0 + OGROUPS[g]) * MM_N],
                        in_=ot[:, :],
                    )
                    gi0 += OGROUPS[g]
                    g += 1

            for i in range(NCHUNK):
                warmup(WARM_IN.get(i, 0))
                ps = ppool.tile([O, MM_N], F32)
                psums[i] = ps
                p3 = ps[:].rearrange("p (r c) -> p r c", r=4)
                # single tap (2,2): lower y only
                nc.tensor.matmul(
                    p3, wt[0:64, 0:64],
                    y1_3[0:64, 4 * i + 2:4 * i + 6, 2:2 + W],
                    start=True, stop=False, skip_group_check=True,
                )
                # row pairs (0,l)+(1,l)
                for l in range(3):
                    nc.tensor.matmul(
                        p3, wt[0:128, 64 + 64 * l:128 + 64 * l],
                        y1_3[0:128, 4 * i:4 * i + 4, l:l + W],
                        start=False, stop=False, skip_group_check=True,
                    )
                # deferred col pairs scheduled for this block, then bias/out
                for i2 in sorted(k for k, v in CP_AT.items() if v == i):
                    colpair_and_bias(i2)
    return nc


def _shard_inputs(x, alpha):
    """Per-core packed slab rows: chunk j < DL is [x rows r0..r1+1 | alpha
    same rows] (the device DMA reads it twice, offset one row, into the two
    partition halves); later chunks are [x_j | alpha_j]. Alpha is fp16 bits
    viewed bf16."""
    maps = []
    for core in range(8):
        b_idx, h = divmod(core, 2)
        r0g = h * HS - 1  # global row of padded row 0
        xs = np.zeros((C, RP + 1, WP), NPBF16)
        als = np.zeros((RP + 1, WP), np.float16)
        lo = max(0, r0g)
        hi = min(H, r0g + RP)
        xs[:, lo - r0g:hi - r0g, 1:1 + W] = x[b_idx, :, lo:hi, :].astype(NPBF16)
        als[lo - r0g:hi - r0g, 1:1 + W] = alpha[b_idx, 0, lo:hi, :]
        alb = als.reshape(-1).view(NPBF16)
        xf = xs.reshape(C, -1)
        xa = np.empty((C, XL_HOST), NPBF16)
        ho = 0
        for j, (r0, r1) in enumerate(EW_ROWS):
            n = EW_N[j]
            m = n + WP if j < DL else n
            r1x = r1 + 1 if j < DL else r1
            xa[:, ho:ho + m] = xf[:, r0 * WP:r1x * WP]
            xa[:, ho + m:ho + 2 * m] = alb[r0 * WP:r1x * WP]
            ho += 2 * m
        maps.append({"xa": xa})
    return maps


def _pack_weights(wt, negate):
    """[O,C,3,3] -> [128, 320] bf16.
    cols 0:64        rows 0:64   = tap (2,2)          (single)
    cols 64+64l:+64  rows c|64+c = taps (0,l)|(1,l)   (row pairs)
    cols 256:320     rows c|64+c = taps (2,0)|(2,1)   (col pair)
    """
    wk = wt.transpose(1, 2, 3, 0)  # [c, k, l, o]
    out = np.zeros((128, 320), np.float32)
    out[:64, 0:64] = wk[:, 2, 2]
    for l in range(3):
        out[:64, 64 + 64 * l:128 + 64 * l] = wk[:, 0, l]
        out[64:, 64 + 64 * l:128 + 64 * l] = wk[:, 1, l]
    out[:64, 256:320] = wk[:, 2, 0]
    out[64:, 256:320] = wk[:, 2, 1]
    if negate:
        out = -out
    return np.ascontiguousarray(out.astype(NPBF16))


def kernel(inputs, alpha, weight, bias, a, b, c):
    x = np.ascontiguousarray(np.asarray(inputs, np.float32))
    al = np.ascontiguousarray(np.asarray(alpha, np.float32))
    wt = np.asarray(weight, np.float32)
    bs = np.asarray(bias, np.float32)
    av, bv, cv = float(a), float(b), float(c)

    key = (av, bv, cv)
    if key not in _cache:
        _cache.clear()
        nc_new = _program(av, bv, cv)
        nc_new.finalize()
        _cache[key] = nc_new
    nc = _cache[key]

    w_packed = _pack_weights(wt, negate=(av < 0))
    b_packed = np.ascontiguousarray(bs.reshape(O, 1))
    in_maps = _shard_inputs(x, al)
    for m in in_maps:
        m["w"] = w_packed
        m["bias"] = b_packed

    res = run_bass_kernel_spmd(nc, in_maps, list(range(8)))

    out = np.empty((B, O, H, W), np.float32)
    for core in range(8):
        b_idx, h = divmod(core, 2)
        out[b_idx, :, h * HS:(h + 1) * HS, :] = (
            res.results[core]["out"].astype(np.float32).reshape(O, HS, W)
        )
    return out


# revision 97
# speedup vs baseline: 1.0428x; 1.0428x over previous
"""ConvSquare Trainium2 kernel.

Math: out = conv2d_3x3(x * poly(alpha), weight) + bias, stride 1, pad 1,
where poly(t) = (a*t + b)*t + c applied to the zero-padded alpha field.
(The reference's unfold/einsum collapses to this because x is zero-padded:
border window positions contribute x=0 regardless of the kernel value.)

Sharding: 8 cores = batch(4) x row-half(2). Each core computes a
[O=64, 64, 128] output slab from a zero-padded [C=64, 66, 130] input slab.

Design (final, ~30.3us modeled vs 51.5us baseline):
- bf16 x/y/w/out, fp16 alpha/z; host converts the output back to f32.
- one DMA per chunk loads [x_j | alpha_j] (bf16 container; alpha is fp16
  bits) from a host-packed image. The first DL chunks are double-loaded:
  a 3-dim AP (outer dim stride WP over the DRAM image) fills partitions
  64-127 with the +1-row-shifted window, so the elementwise pass below
  produces y AND its row-shifted replica at once (engine cost depends
  only on free size). Later chunks get the replica via an SBUF shift copy.
- poly via completed square: kal = sgn*(s*al + t)^2 + d -> one ACT Square
  (z, fp16) + one DVE scalar_tensor_tensor (y = (z -/+ d)*x, bf16); the
  sign is folded into host-negated weights. Two-op feed chain per chunk.
- 5 matmuls per 512-col output chunk instead of 9 naive: 1 single (2,2)
  + 3 row pairs ((0,l)+(1,l) via y1's shifted upper half) + 1 col pair
  ((2,0)+(2,1) via y2 = [y ; y shifted one column]).
- queue balance: inputs/outputs/shift/y2-lower on SP's HWDGE queue, ycol
  and the last output group on the Pool SWDGE queue (HWDGE gen ~0.63us
  and Pool gen ~1.0us serialize per queue; balance beats either alone).
- a p-state bridge matmul on a const tile at t~0.8us starts the PE ramp
  clock early and const-based warmups keep PE busy through the prologue,
  so all real matmuls are costed at full clock (idle-start would halve
  the clock for the first ~3us of visits).
- a dummy activation at t=0 pins LoadActFuncSet off the critical path.
- bias-add + PSUM->SBUF downcast on ACT (Identity + bias AP); the last
  chunk's bias runs on DVE (tracks the stop-matmul within ~100ns).
- output DMA groups of [6,6,2,1,1] chunks so the final transfer after
  the last matmul is minimal.
"""

import sys

import ml_dtypes
import numpy as np

sys.path.insert(0, "/opt/trn_rl_repo")

import concourse.bass as bass
import concourse.mybir as mybir
from concourse.bass_utils import run_bass_kernel_spmd
from concourse.tile import TileContext

F32 = mybir.dt.float32
BF16 = mybir.dt.bfloat16
FP16 = mybir.dt.float16
NPBF16 = ml_dtypes.bfloat16

B, C, O, H, W = 4, 64, 64, 128, 128
HS = 64  # output rows per core
RP = HS + 2  # padded input rows (66)
WP = W + 2  # padded cols (130)
FREE = RP * WP  # 8580
EW_ROWS = [(0, 6), (6, 11), (11, 16), (16, 22), (22, 30), (30, 38), (38, 46),
           (46, 56), (56, 66)]
EW_N = [(r1 - r0) * WP for r0, r1 in EW_ROWS]
DL = 4  # first DL chunks are double-loaded (128 partitions, built-in shift)
# sbuf region len per chunk: double-loaded = [x|x-extra-row|al], else [x|al]
EW_L = [2 * n + WP if j < DL else 2 * n for j, n in enumerate(EW_N)]
EW_HOST = [2 * (n + WP) if j < DL else 2 * n for j, n in enumerate(EW_N)]
XL = sum(EW_L)
XL_HOST = sum(EW_HOST)
NCHUNK = 16  # matmul blocks (4 out rows each)
MM_N = 4 * W  # 512
OGROUPS = [6, 6, 2, 1, 1]  # output DMA groups (psum chunks each)
# col-pair of block i is emitted in block CP_AT[i]: deferred two blocks
# early on (its y2 copies land late), in-block once the copies lead the PE
CP_AT = {i: (i + 2 if i < 6 else (i + 1 if i < 10 else i))
         for i in range(NCHUNK)}
WARM0 = 2  # warmup matmuls (the scheduler hoists them to the prologue)
WARM_IN = {}

_cache: dict = {}


def _program(av: float, bv: float, cv: float) -> bass.Bass:
    from concourse.bacc import Bacc

    nc = Bacc()
    xa_h = nc.dram_tensor("xa", [64, XL_HOST], BF16, kind="ExternalInput")
    w_h = nc.dram_tensor("w", [128, 320], BF16, kind="ExternalInput")
    bias_h = nc.dram_tensor("bias", [O, 1], F32, kind="ExternalInput")
    out_h = nc.dram_tensor("out", [O, HS * W], BF16, kind="ExternalOutput")

    # poly(alpha) = a*al^2 + b*al + c as sgn*(s*al + t)^2 + d
    if av == 0.0:
        # linear fallback: z = b*al + c (ACT Copy), y = (z + 0) * x
        s, t, d = bv, cv, 0.0
        op0 = mybir.AluOpType.add
    elif av < 0:
        s = (-av) ** 0.5
        t = -bv / (2 * s)
        d = cv - bv * bv / (4 * av)
        op0 = mybir.AluOpType.subtract  # y = (z - d)*x = -(kal)*x, w negated
    else:
        s = av ** 0.5
        t = bv / (2 * s)
        d = cv - bv * bv / (4 * av)
        op0 = mybir.AluOpType.add  # y = (z + d)*x

    def mk_ap(base, offset, dims):
        return bass.AP(tensor=base.tensor, offset=offset, ap=dims)

    with TileContext(nc) as tc:
        with (
            tc.tile_pool(name="const", bufs=1) as cpool,
            tc.tile_pool(name="work", bufs=1) as wpool,
            tc.tile_pool(name="outs", bufs=5) as opool,
            tc.tile_pool(name="psum", bufs=7, space="PSUM") as ppool,
            tc.tile_pool(name="wpsum", bufs=1, space="PSUM") as wppool,
        ):
            wt = cpool.tile([128, 320], BF16)
            bt = cpool.tile([O, 1], F32)
            tc_b = cpool.tile([128, 1], F32)
            nc.gpsimd.memset(tc_b[:, :], t)
            cb2 = cpool.tile([128, 1], BF16)
            nc.gpsimd.memset(cb2[:, :], 0.5)
            zdum = cpool.tile([128, 1], F32)
            # dummy op: pins LoadActFuncSet at t~0, off the critical path
            nc.scalar.activation(
                zdum[:, :], tc_b[:, :],
                mybir.ActivationFunctionType.Square, bias=tc_b[:, 0:1],
                scale=1.0,
            )

            xa = wpool.tile([128, XL], BF16)
            zt = wpool.tile([128, FREE], FP16)
            y1 = wpool.tile([128, FREE], BF16)
            y2 = wpool.tile([128, FREE], BF16)

            # SP/HWDGE queue: all loads up-front (no waits -> no blocking)
            so_j = [0]
            for L in EW_L:
                so_j.append(so_j[-1] + L)
            ho_j = [0]
            for L in EW_HOST:
                ho_j.append(ho_j[-1] + L)
            for j in range(len(EW_ROWS)):
                if j < DL:
                    # 3-dim AP: outer dim of stride WP loads partitions
                    # 64-127 with the +1-row shifted window
                    nc.sync.dma_start(
                        out=xa[:, so_j[j]:so_j[j + 1]],
                        in_=mk_ap(xa_h[:, :], ho_j[j],
                                  [[WP, 2], [XL_HOST, 64], [1, EW_L[j]]]),
                    )
                else:
                    nc.sync.dma_start(
                        out=xa[0:64, so_j[j]:so_j[j + 1]],
                        in_=xa_h[:, ho_j[j]:ho_j[j + 1]],
                    )
                if j == 1:
                    nc.sync.dma_start(out=wt[:, :], in_=w_h[:, :])
                    nc.sync.dma_start(out=bt[:, :], in_=bias_h[:, :])

            wps = wppool.tile([128, 512], F32)

            # p-state bridge: one long f32 matmul on the const tile (ready at
            # ~0.5us) marks the PE busy-period start early, so the real
            # matmuls are costed at full clock. f32 runs 4 cycles/row; the
            # stride-0 rhs makes it 512 rows from a [128,1] tile.
            nc.tensor.matmul(
                wps[0:1, :], tc_b[:, 0:1], tc_b[:, 0:1].to_broadcast((128, 512)),
                start=True, stop=True, skip_group_check=True,
            )
            nc.tensor.matmul(
                wps[0:1, 0:256], tc_b[:, 0:1],
                tc_b[:, 0:1].to_broadcast((128, 256)),
                start=True, stop=True, skip_group_check=True,
            )

            def warmup(k):
                # const-tile operands: no input dependency, can run at t~1us
                for _ in range(k):
                    nc.tensor.matmul(
                        wps[0:1, 0:320], cb2[:, 0:1],
                        cb2[:, 0:1].to_broadcast((128, 320)),
                        start=True, stop=True, skip_group_check=True,
                    )

            def emit_poly(j, r0, r1):
                R0 = EW_ROWS[j][0]
                n = EW_N[j]
                off = (r0 - R0) * WP
                m = (r1 - r0) * WP
                sl = slice(r0 * WP, r1 * WP)
                if j < DL:
                    np_, al0 = 128, so_j[j] + n + WP
                else:
                    np_, al0 = 64, so_j[j] + n
                x_ap = xa[0:np_, so_j[j] + off:so_j[j] + off + m]
                al_ap = xa[0:np_, al0 + off:al0 + off + m].bitcast(FP16)
                # z = (s*al + t)^2   (ACT Square, fp16); linear for a=0
                if av == 0.0:
                    nc.scalar.activation(
                        zt[0:np_, sl], al_ap,
                        mybir.ActivationFunctionType.Copy, bias=t, scale=s,
                    )
                else:
                    nc.scalar.activation(
                        zt[0:np_, sl], al_ap,
                        mybir.ActivationFunctionType.Square,
                        bias=tc_b[0:np_, 0:1], scale=s,
                    )
                # y = (z -/+ d) * x  (DVE scalar_tensor_tensor, bf16)
                # double-loaded chunks produce y and its row-shifted replica
                # in the same pass (partitions 64-127)
                nc.vector.scalar_tensor_tensor(
                    out=y1[0:np_, sl], in0=zt[0:np_, sl], in1=x_ap, scalar=d,
                    op0=op0, op1=mybir.AluOpType.mult,
                )

            def emit_shift(j):
                r0, r1 = EW_ROWS[j]
                a0, b0 = r0 * WP, r1 * WP
                if j >= DL:
                    # +1-row shift onto y1 upper (SP queue)
                    c0, c1 = max(0, a0 - WP), b0 - WP
                    nc.sync.dma_start(
                        out=y1[64:128, c0:c1], in_=y1[0:64, c0 + WP:c1 + WP]
                    )

            def emit_y2(r0, r1):
                # SP queue: y2 = [y ; y shifted one column]
                a0, b0 = r0 * WP, r1 * WP
                nc.sync.dma_start(out=y2[0:64, a0:b0], in_=y1[0:64, a0:b0])
                d0, d1 = max(0, a0 - 1), b0 - 1
                nc.gpsimd.dma_start(
                    out=y2[64:128, d0:d1], in_=y1[0:64, d0 + 1:d1 + 1]
                )

            def emit_chunk(j):
                emit_poly(j, *EW_ROWS[j])
                emit_shift(j)
                emit_y2(*EW_ROWS[j])

            y1_3 = y1[:].rearrange("p (r c) -> p r c", r=RP)
            y2_3 = y2[:].rearrange("p (r c) -> p r c", r=RP)

            # chunk 0 split: rows 2-6 first (block 0's single reads rows 2-5)
            emit_poly(0, 2, 6)
            emit_poly(0, 0, 2)
            emit_y2(*EW_ROWS[0])
            warmup(WARM0)
            for k in range(1, len(EW_ROWS)):
                emit_chunk(k)

            psums = {}
            gi0 = 0
            g = 0
            ot = None

            def colpair_and_bias(i2):
                nonlocal gi0, g, ot
                ps2 = psums.pop(i2)
                nc.tensor.matmul(
                    ps2[:].rearrange("p (r c) -> p r c", r=4),
                    wt[0:128, 256:320],
                    y2_3[0:128, 4 * i2 + 2:4 * i2 + 6, 0:W],
                    start=False, stop=True, skip_group_check=True,
                )
                if i2 == gi0:
                    ot = opool.tile([O, OGROUPS[g] * MM_N], BF16)
                oc = (i2 - gi0) * MM_N
                if i2 >= NCHUNK - 1:
                    # tail-critical biases on DVE: idle there, and it starts
                    # within ~100ns of the stop-matmul firing
                    nc.vector.tensor_scalar(
                        out=ot[:, oc:oc + MM_N], in0=ps2[:, :],
                        scalar1=bt[:, 0:1], scalar2=None,
                        op0=mybir.AluOpType.add,
                    )
                else:
                    nc.scalar.activation(
                        ot[:, oc:oc + MM_N], ps2[:, :],
                        mybir.ActivationFunctionType.Identity,
                        bias=bt[:, 0:1], scale=1.0,
                    )
                if i2 - gi0 == OGROUPS[g] - 1:
                    # last two groups issue from the (idle) ACT/DVE queues so
                    # the earlier groups' DMAs can't head-of-line block them
                    eng = nc.sync
                    eng.dma_start(
                        out=out_h[:, gi0 * MM_N:(gi0 + OGROUPS[g]) * MM_N],
                        in_=ot[:, :],
                    )
                    gi0 += OGROUPS[g]
                    g += 1

            for i in range(NCHUNK):
                warmup(WARM_IN.get(i, 0))
                ps = ppool.tile([O, MM_N], F32)
                psums[i] = ps
                p3 = ps[:].rearrange("p (r c) -> p r c", r=4)
                # single tap (2,2): lower y only
                nc.tensor.matmul(
                    p3, wt[0:64, 0:64],
                    y1_3[0:64, 4 * i + 2:4 * i + 6, 2:2 + W],
                    start=True, stop=False, skip_group_check=True,
                )
                # row pairs (0,l)+(1,l)
                for l in range(3):
                    nc.tensor.matmul(
                        p3, wt[0:128, 64 + 64 * l:128 + 64 * l],
                        y1_3[0:128, 4 * i:4 * i + 4, l:l + W],
                        start=False, stop=False, skip_group_check=True,
                    )
                # deferred col pairs scheduled for this block, then bias/out
                for i2 in sorted(k for k, v in CP_AT.items() if v == i):
                    colpair_and_bias(i2)
    return nc


def _shard_inputs(x, alpha):
    """Per-core packed slab rows: chunk j < DL is [x rows r0..r1+1 | alpha
    same rows] (the device DMA reads it twice, offset one row, into the two
    partition halves); later chunks are [x_j | alpha_j]. Alpha is fp16 bits
    viewed bf16."""
    maps = []
    for core in range(8):
        b_idx, h = divmod(core, 2)
        r0g = h * HS - 1  # global row of padded row 0
        xs = np.zeros((C, RP + 1, WP), NPBF16)
        als = np.zeros((RP + 1, WP), np.float16)
        lo = max(0, r0g)
        hi = min(H, r0g + RP)
        xs[:, lo - r0g:hi - r0g, 1:1 + W] = x[b_idx, :, lo:hi, :].astype(NPBF16)
        als[lo - r0g:hi - r0g, 1:1 + W] = alpha[b_idx, 0, lo:hi, :]
        alb = als.reshape(-1).view(NPBF16)
        xf = xs.reshape(C, -1)
        xa = np.empty((C, XL_HOST), NPBF16)
        ho = 0
        for j, (r0, r1) in enumerate(EW_ROWS):
            n = EW_N[j]
            m = n + WP if j < DL else n
            r1x = r1 + 1 if j < DL else r1
            xa[:, ho:ho + m] = xf[:, r0 * WP:r1x * WP]
            xa[:, ho + m:ho + 2 * m] = alb[r0 * WP:r1x * WP]
            ho += 2 * m
        maps.append({"xa": xa})
    return maps


def _pack_weights(wt, negate):
    """[O,C,3,3] -> [128, 320] bf16.
    cols 0:64        rows 0:64   = tap (2,2)          (single)
    cols 64+64l:+64  rows c|64+c = taps (0,l)|(1,l)   (row pairs)
    cols 256:320     rows c|64+c = taps (2,0)|(2,1)   (col pair)
    """
    wk = wt.transpose(1, 2, 3, 0)  # [c, k, l, o]
    out = np.zeros((128, 320), np.float32)
    out[:64, 0:64] = wk[:, 2, 2]
    for l in range(3):
        out[:64, 64 + 64 * l:128 + 64 * l] = wk[:, 0, l]
        out[64:, 64 + 64 * l:128 + 64 * l] = wk[:, 1, l]
    out[:64, 256:320] = wk[:, 2, 0]
    out[64:, 256:320] = wk[:, 2, 1]
    if negate:
        out = -out
    return np.ascontiguousarray(out.astype(NPBF16))


def kernel(inputs, alpha, weight, bias, a, b, c):
    x = np.ascontiguousarray(np.asarray(inputs, np.float32))
    al = np.ascontiguousarray(np.asarray(alpha, np.float32))
    wt = np.asarray(weight, np.float32)
    bs = np.asarray(bias, np.float32)
    av, bv, cv = float(a), float(b), float(c)

    key = (av, bv, cv)
    if key not in _cache:
        _cache.clear()
        nc_new = _program(av, bv, cv)
        nc_new.finalize()
        _cache[key] = nc_new
    nc = _cache[key]

    w_packed = _pack_weights(wt, negate=(av < 0))
    b_packed = np.ascontiguousarray(bs.reshape(O, 1))
    in_maps = _shard_inputs(x, al)
    for m in in_maps:
        m["w"] = w_packed
        m["bias"] = b_packed

    res = run_bass_kernel_spmd(nc, in_maps, list(range(8)))

    out = np.empty((B, O, H, W), np.float32)
    for core in range(8):
        b_idx, h = divmod(core, 2)
        out[b_idx, :, h * HS:(h + 1) * HS, :] = (
            res.results[core]["out"].astype(np.float32).reshape(O, HS, W)
        )
    return out
